# revision 3
# baseline (speedup 1.0000x reference)
"""Chamfer distance kernel for Trainium2 (8 NeuronCores, Bass/Tile).

Problem: x [4, 8192, 3], y [4, 8192, 3] f32.
  d[b,i,j] = ||x[b,i] - y[b,j]||^2
  out = mean_b mean_i min_j d  +  mean_b mean_j min_i d   (scalar f32)

Sharding: core k handles batch b = k//2, half h = k%2 of x's N dimension.
Each core computes d for its [4096 x-rows] x [all 8192 y] block via an
augmented K=5 matmul on the TensorEngine:
    lhsT rows = [x^2, 1, x0, x1, x2],  rhs rows = [1, y^2, -2*y0, -2*y1, -2*y2]
so PSUM holds exact f32 distances. ScalarE extracts PSUM->SBUF with an
f32->fp16 cast; VectorE keeps a running row-min (free-axis fold) and a
running col-min tile [128, 8192]. Col-min partition reduction happens on
device via PE transposes + DVE reduce. The host only combines tiny per-core
outputs ([128,1] row-min sums and [128,64] col-min blocks).
"""

import numpy as np
from contextlib import ExitStack

import concourse.bass as bass
import concourse.bacc as bacc
import concourse.tile as tile
from concourse import mybir
from concourse.bass_utils import run_bass_kernel_spmd

B, N, M, D = 4, 8192, 8192, 3
NCORES = 8
HALF = N // 2            # x rows per core
NIT = HALF // 128        # 32 i-tiles
STW = 2048               # supertile width (4 PSUM banks)
NST = M // STW           # 4 supertiles per i-tile
NBLK = M // 128          # 64 col-min blocks
F32 = mybir.dt.float32
F16 = mybir.dt.float16
AX = mybir.AxisListType.X
MIN = mybir.AluOpType.min
ADD = mybir.AluOpType.add


def _build():
    nc = bacc.Bacc("TRN2", target_bir_lowering=False, num_devices=NCORES)
    xT = nc.declare_dram_parameter("xT", [3, HALF], F32, isOutput=False)
    yT = nc.declare_dram_parameter("yT", [3, M], F32, isOutput=False)
    xq = nc.declare_dram_parameter("xq", [128, NIT * 3], F32, isOutput=False)
    yq = nc.declare_dram_parameter("yq", [128, NBLK * 3], F32, isOutput=False)
    idf = nc.declare_dram_parameter("idf", [128, 128], F32, isOutput=False)
    idh = nc.declare_dram_parameter("idh", [128, 128], F16, isOutput=False)
    rm_out = nc.declare_dram_parameter("rm_out", [128, 1], F32, isOutput=True)
    cm_out = nc.declare_dram_parameter("cm_out", [128, NBLK], F32, isOutput=True)

    with ExitStack() as ctx:
        tc = ctx.enter_context(tile.TileContext(nc))
        persist = ctx.enter_context(tc.tile_pool(name="persist", bufs=1))
        PHI = persist.tile([5, HALF], F32)
        PSI = persist.tile([5, M], F32)
        cm = persist.tile([128, M], F16)
        rm_cols = persist.tile([128, NIT], F32)
        rm_sums = persist.tile([128, 1], F32)
        cmb = persist.tile([128, NBLK], F32)
        identf = persist.tile([128, 128], F32)
        identh = persist.tile([128, 128], F16)

        # Row layout (all compute-written rows start at partition 0):
        #   PHI rows: 0-2 = x,    3 = ones, 4 = x^2
        #   PSI rows: 0-2 = -2*y, 3 = y^2,  4 = ones
        nc.sync.dma_start(out=identf, in_=idf[:, :])
        nc.sync.dma_start(out=identh, in_=idh[:, :])
        nc.sync.dma_start(out=PHI[0:3, :], in_=xT[:, :])
        nc.vector.memset(cm, 60000.0)

        # ---- prep: squared norms into PHI[4] (x^2) and PSI[3] (y^2) ----
        with tc.tile_pool(name="prep", bufs=1) as prep, \
             tc.tile_pool(name="prep_ps", bufs=1, space="PSUM") as prep_ps:
            ones_stage = prep.tile([1, M], F32)
            nc.vector.memset(ones_stage, 1.0)
            nc.sync.dma_start(out=PHI[3:4, :], in_=ones_stage[0:1, 0:HALF])
            nc.sync.dma_start(out=PSI[4:5, :], in_=ones_stage)
            yst = prep.tile([3, M], F32)
            nc.sync.dma_start(out=yst, in_=yT[:, :])
            nc.vector.tensor_scalar_mul(PSI[0:3, :], yst, -2.0)
            xq_t = prep.tile([128, NIT * 3], F32)
            yq_t = prep.tile([128, NBLK * 3], F32)
            nc.sync.dma_start(out=xq_t, in_=xq[:, :])
            nc.sync.dma_start(out=yq_t, in_=yq[:, :])
            sqx = prep.tile([128, NIT * 3], F32)
            sqy = prep.tile([128, NBLK * 3], F32)
            nc.scalar.activation(sqx, xq_t, mybir.ActivationFunctionType.Square)
            nc.scalar.activation(sqy, yq_t, mybir.ActivationFunctionType.Square)
            x2q = prep.tile([128, NIT], F32)
            y2q = prep.tile([128, NBLK], F32)
            nc.vector.tensor_reduce(
                out=x2q, in_=sqx.rearrange("p (t d) -> p t d", d=3), axis=AX, op=ADD
            )
            nc.vector.tensor_reduce(
                out=y2q, in_=sqy.rearrange("p (t d) -> p t d", d=3), axis=AX, op=ADD
            )
            # transpose [128, T] -> [T, 128] so free dim becomes the point idx
            x2ps = prep_ps.tile([NIT, 128], F32)
            y2ps = prep_ps.tile([NBLK, 128], F32)
            nc.tensor.transpose(x2ps, x2q, identf)
            nc.tensor.transpose(y2ps, y2q, identf)
            x2t = prep.tile([NIT, 128], F32)
            y2t = prep.tile([NBLK, 128], F32)
            nc.scalar.copy(x2t, x2ps)
            nc.scalar.copy(y2t, y2ps)
            # relayout [T, 128] -> single row [1, T*128] (j = t*128 + p)
            nc.sync.dma_start(
                out=PHI[4:5, :].rearrange("a (t p) -> a t p", p=128), in_=x2t
            )
            nc.sync.dma_start(
                out=PSI[3:4, :].rearrange("a (t p) -> a t p", p=128), in_=y2t
            )

        # ---- main loop ----
        with tc.tile_pool(name="mm_ps", bufs=2, space="PSUM") as mm_ps, \
             tc.tile_pool(name="ext", bufs=6) as ext:
            for it in range(NIT):
                phi_it = PHI[:, it * 128:(it + 1) * 128]
                rm_run = None
                for st in range(NST):
                    ps = mm_ps.tile([128, STW], F32, tag="ps", name=f"ps_{it}_{st}")
                    for c in range(STW // 512):
                        j0 = st * STW + c * 512
                        nc.tensor.matmul(
                            ps[:, c * 512:(c + 1) * 512],
                            phi_it,
                            PSI[:, j0:j0 + 512],
                            start=True,
                            stop=True,
                        )
                    e = ext.tile([128, STW], F16, tag="e", name=f"e_{it}_{st}")
                    nc.scalar.copy(e, ps)
                    # running col-min (elementwise, fp16 2x mode)
                    cs = cm[:, st * STW:(st + 1) * STW]
                    nc.vector.tensor_tensor(out=cs, in0=cs, in1=e, op=MIN)
                    # running row-min across supertiles
                    if st == 0:
                        rm_run = e
                    else:
                        nc.vector.tensor_tensor(out=rm_run, in0=rm_run, in1=e, op=MIN)
                # fold row-min 2048 -> 256, then reduce
                nc.vector.tensor_tensor(
                    out=rm_run[:, 0:1024], in0=rm_run[:, 0:1024],
                    in1=rm_run[:, 1024:2048], op=MIN,
                )
                nc.vector.tensor_tensor(
                    out=rm_run[:, 0:512], in0=rm_run[:, 0:512],
                    in1=rm_run[:, 512:1024], op=MIN,
                )
                nc.vector.tensor_tensor(
                    out=rm_run[:, 0:256], in0=rm_run[:, 0:256],
                    in1=rm_run[:, 256:512], op=MIN,
                )
                nc.vector.tensor_reduce(
                    out=rm_cols[:, it:it + 1], in_=rm_run[:, 0:256], axis=AX, op=MIN
                )

        # ---- tails ----
        nc.vector.tensor_reduce(out=rm_sums, in_=rm_cols, axis=AX, op=ADD)
        nc.sync.dma_start(out=rm_out[:, :], in_=rm_sums)

        with tc.tile_pool(name="tail_ps", bufs=2, space="PSUM") as tail_ps:
            for bg in range(NBLK // 4):
                pt = tail_ps.tile([128, 512], F16, tag="pt", name=f"pt_{bg}")
                for q in range(4):
                    blk = bg * 4 + q
                    nc.tensor.transpose(
                        pt[:, q * 128:(q + 1) * 128],
                        cm[:, blk * 128:(blk + 1) * 128],
                        identh,
                    )
                nc.vector.tensor_reduce(
                    out=cmb[:, bg * 4:(bg + 1) * 4],
                    in_=pt.rearrange("p (q f) -> p q f", f=128),
                    axis=AX,
                    op=MIN,
                )
        nc.sync.dma_start(out=cm_out[:, :], in_=cmb)

    nc.compile()
    return nc


_NC = None


def _get_nc():
    global _NC
    if _NC is None:
        _NC = _build()
    return _NC


def _in_maps(x, y):
    idf = np.eye(128, dtype=np.float32)
    idh = np.eye(128, dtype=np.float16)
    maps = []
    for k in range(NCORES):
        b, h = divmod(k, 2)
        xs = x[b, h * HALF:(h + 1) * HALF]    # [4096, 3]
        ys = y[b]                              # [8192, 3]
        maps.append({
            "xT": np.ascontiguousarray(xs.T),
            "yT": np.ascontiguousarray(ys.T),
            "xq": np.ascontiguousarray(
                xs.reshape(NIT, 128, 3).transpose(1, 0, 2).reshape(128, NIT * 3)
            ),
            "yq": np.ascontiguousarray(
                ys.reshape(NBLK, 128, 3).transpose(1, 0, 2).reshape(128, NBLK * 3)
            ),
            "idf": idf,
            "idh": idh,
        })
    return maps


def _postprocess(results):
    rm_total = 0.0
    cham_y_total = 0.0
    vecs = []
    for k in range(NCORES):
        rm_total += float(results[k]["rm_out"].astype(np.float64).sum())
        vecs.append(results[k]["cm_out"].T.reshape(M))  # vec[j], j = blk*128+p
    for b in range(B):
        m = np.minimum(vecs[2 * b], vecs[2 * b + 1])
        cham_y_total += float(m.astype(np.float64).sum())
    out = rm_total / (B * N) + cham_y_total / (B * M)
    return np.float32(out)


def kernel(x, y):
    x = np.asarray(x, dtype=np.float32)
    y = np.asarray(y, dtype=np.float32)
    nc = _get_nc()
    res = run_bass_kernel_spmd(nc, _in_maps(x, y), core_ids=list(range(NCORES)))
    return _postprocess(res.results)


# revision 7
# speedup vs baseline: 2.1171x; 2.1171x over previous
"""Chamfer distance kernel for Trainium2 (8 NeuronCores, Bass/Tile).

Problem: x [4, 8192, 3], y [4, 8192, 3] f32.
  d[b,i,j] = ||x[b,i] - y[b,j]||^2
  out = mean_b mean_i min_j d  +  mean_b mean_j min_i d   (scalar f32)

Sharding: core k handles batch b = k//2, half h = k%2 of x's N dimension.
Each core computes d for its [4096 x-rows] x [all 8192 y] block via an
augmented K=5 matmul on the TensorEngine:
    lhsT rows = [x^2, 1, x0, x1, x2],  rhs rows = [1, y^2, -2*y0, -2*y1, -2*y2]
so PSUM holds exact f32 distances. ScalarE extracts PSUM->SBUF with an
f32->fp16 cast; VectorE keeps a running row-min (free-axis fold) and a
running col-min tile [128, 8192]. Col-min partition reduction happens on
device via PE transposes + DVE reduce. The host only combines tiny per-core
outputs ([128,1] row-min sums and [128,64] col-min blocks).
"""

import numpy as np
from contextlib import ExitStack

import concourse.bass as bass
import concourse.bacc as bacc
import concourse.tile as tile
from concourse import mybir
from concourse.bass_utils import run_bass_kernel_spmd

B, N, M, D = 4, 8192, 8192, 3
NCORES = 8
HALF = N // 2            # x rows per core
NIT = HALF // 128        # 32 i-tiles
STW = 2048               # supertile width (4 PSUM banks)
NST = M // STW           # 4 supertiles per i-tile
NBLK = M // 128          # 64 col-min blocks
F32 = mybir.dt.float32
F16 = mybir.dt.float16
AX = mybir.AxisListType.X
MIN = mybir.AluOpType.min
ADD = mybir.AluOpType.add


def _build():
    nc = bacc.Bacc("TRN2", target_bir_lowering=False, num_devices=NCORES)
    xT = nc.declare_dram_parameter("xT", [3, HALF], F32, isOutput=False)
    yT = nc.declare_dram_parameter("yT", [3, M], F32, isOutput=False)
    xq = nc.declare_dram_parameter("xq", [128, NIT * 3], F32, isOutput=False)
    yq = nc.declare_dram_parameter("yq", [128, NBLK * 3], F32, isOutput=False)
    idf = nc.declare_dram_parameter("idf", [128, 128], F32, isOutput=False)
    idh = nc.declare_dram_parameter("idh", [128, 128], F16, isOutput=False)
    rm_out = nc.declare_dram_parameter("rm_out", [128, 1], F32, isOutput=True)
    cm_out = nc.declare_dram_parameter("cm_out", [128, NBLK], F32, isOutput=True)

    with ExitStack() as ctx:
        tc = ctx.enter_context(tile.TileContext(nc))
        persist = ctx.enter_context(tc.tile_pool(name="persist", bufs=1))
        PHI = persist.tile([5, HALF], F32)
        PSI = persist.tile([5, M], F32)
        cm = persist.tile([128, M], F16)
        rm_cols = persist.tile([128, NIT], F32)
        rm_sums = persist.tile([128, 1], F32)
        cmb = persist.tile([128, NBLK], F32)
        identf = persist.tile([128, 128], F32)
        identh = persist.tile([128, 128], F16)

        # Row layout (all compute-written rows start at partition 0):
        #   PHI rows: 0-2 = x,    3 = ones, 4 = x^2
        #   PSI rows: 0-2 = -2*y, 3 = y^2,  4 = ones
        nc.sync.dma_start(out=identf, in_=idf[:, :])
        nc.sync.dma_start(out=identh, in_=idh[:, :])
        nc.sync.dma_start(out=PHI[0:3, :], in_=xT[:, :])
        nc.vector.memset(cm, 60000.0)

        # ---- prep: squared norms into PHI[4] (x^2) and PSI[3] (y^2) ----
        with tc.tile_pool(name="prep", bufs=1) as prep, \
             tc.tile_pool(name="prep_ps", bufs=1, space="PSUM") as prep_ps:
            ones_stage = prep.tile([1, M], F32)
            nc.vector.memset(ones_stage, 1.0)
            nc.sync.dma_start(out=PHI[3:4, :], in_=ones_stage[0:1, 0:HALF])
            nc.sync.dma_start(out=PSI[4:5, :], in_=ones_stage)
            yst = prep.tile([3, M], F32)
            nc.sync.dma_start(out=yst, in_=yT[:, :])
            nc.vector.tensor_scalar_mul(PSI[0:3, :], yst, -2.0)
            xq_t = prep.tile([128, NIT * 3], F32)
            yq_t = prep.tile([128, NBLK * 3], F32)
            nc.sync.dma_start(out=xq_t, in_=xq[:, :])
            nc.sync.dma_start(out=yq_t, in_=yq[:, :])
            sqx = prep.tile([128, NIT * 3], F32)
            sqy = prep.tile([128, NBLK * 3], F32)
            nc.scalar.activation(sqx, xq_t, mybir.ActivationFunctionType.Square)
            nc.scalar.activation(sqy, yq_t, mybir.ActivationFunctionType.Square)
            x2q = prep.tile([128, NIT], F32)
            y2q = prep.tile([128, NBLK], F32)
            nc.vector.tensor_reduce(
                out=x2q, in_=sqx.rearrange("p (t d) -> p t d", d=3), axis=AX, op=ADD
            )
            nc.vector.tensor_reduce(
                out=y2q, in_=sqy.rearrange("p (t d) -> p t d", d=3), axis=AX, op=ADD
            )
            # transpose [128, T] -> [T, 128] so free dim becomes the point idx
            x2ps = prep_ps.tile([NIT, 128], F32)
            y2ps = prep_ps.tile([NBLK, 128], F32)
            nc.tensor.transpose(x2ps, x2q, identf)
            nc.tensor.transpose(y2ps, y2q, identf)
            x2t = prep.tile([NIT, 128], F32)
            y2t = prep.tile([NBLK, 128], F32)
            nc.scalar.copy(x2t, x2ps)
            nc.scalar.copy(y2t, y2ps)
            # relayout [T, 128] -> single row [1, T*128] (j = t*128 + p)
            nc.sync.dma_start(
                out=PHI[4:5, :].rearrange("a (t p) -> a t p", p=128), in_=x2t
            )
            nc.sync.dma_start(
                out=PSI[3:4, :].rearrange("a (t p) -> a t p", p=128), in_=y2t
            )

        # ---- main loop ----
        with tc.tile_pool(name="mm_ps", bufs=2, space="PSUM") as mm_ps, \
             tc.tile_pool(name="ext", bufs=6) as ext:
            for it in range(NIT):
                phi_it = PHI[:, it * 128:(it + 1) * 128]
                rm_run = None
                for st in range(NST):
                    ps = mm_ps.tile([128, STW], F32, tag="ps", name=f"ps_{it}_{st}")
                    for c in range(STW // 512):
                        j0 = st * STW + c * 512
                        nc.tensor.matmul(
                            ps[:, c * 512:(c + 1) * 512],
                            phi_it,
                            PSI[:, j0:j0 + 512],
                            start=True,
                            stop=True,
                        )
                    e = ext.tile([128, STW], F16, tag="e", name=f"e_{it}_{st}")
                    nc.scalar.copy(e, ps)
                    # running col-min (elementwise, fp16 2x mode)
                    cs = cm[:, st * STW:(st + 1) * STW]
                    nc.vector.tensor_tensor(out=cs, in0=cs, in1=e, op=MIN)
                    # running row-min across supertiles
                    if st == 0:
                        rm_run = e
                    else:
                        nc.vector.tensor_tensor(out=rm_run, in0=rm_run, in1=e, op=MIN)
                # fold row-min 2048 -> 256, then reduce
                nc.vector.tensor_tensor(
                    out=rm_run[:, 0:1024], in0=rm_run[:, 0:1024],
                    in1=rm_run[:, 1024:2048], op=MIN,
                )
                nc.vector.tensor_tensor(
                    out=rm_run[:, 0:512], in0=rm_run[:, 0:512],
                    in1=rm_run[:, 512:1024], op=MIN,
                )
                nc.vector.tensor_tensor(
                    out=rm_run[:, 0:256], in0=rm_run[:, 0:256],
                    in1=rm_run[:, 256:512], op=MIN,
                )
                nc.vector.tensor_reduce(
                    out=rm_cols[:, it:it + 1], in_=rm_run[:, 0:256], axis=AX, op=MIN
                )

        # ---- tails ----
        nc.vector.tensor_reduce(out=rm_sums, in_=rm_cols, axis=AX, op=ADD)
        nc.sync.dma_start(out=rm_out[:, :], in_=rm_sums)

        with tc.tile_pool(name="tail_ps", bufs=2, space="PSUM") as tail_ps:
            for bg in range(NBLK // 4):
                pt = tail_ps.tile([128, 512], F16, tag="pt", name=f"pt_{bg}")
                for q in range(4):
                    blk = bg * 4 + q
                    nc.tensor.transpose(
                        pt[:, q * 128:(q + 1) * 128],
                        cm[:, blk * 128:(blk + 1) * 128],
                        identh,
                    )
                nc.vector.tensor_reduce(
                    out=cmb[:, bg * 4:(bg + 1) * 4],
                    in_=pt.rearrange("p (q f) -> p q f", f=128),
                    axis=AX,
                    op=MIN,
                )
        nc.sync.dma_start(out=cm_out[:, :], in_=cmb)

    nc.compile()
    return nc


_NC = None


def _get_nc():
    global _NC
    if _NC is None:
        _NC = _build()
    return _NC


_RUNNER = None


def _get_runner():
    """Build the Bass program once and return a cached jitted 8-core runner.

    Mirrors bass2jax.run_bass_via_pjrt's multi-core path, but keeps the jitted
    shard_map callable alive so repeated kernel() calls skip XLA re-tracing.
    """
    global _RUNNER
    if _RUNNER is not None:
        return _RUNNER
    import jax
    from jax.sharding import Mesh, PartitionSpec
    from jax.experimental.shard_map import shard_map
    from concourse import mybir as mb
    from concourse import bass2jax

    nc = _get_nc()
    bass2jax.install_neuronx_cc_hook()
    partition_name = (
        nc.partition_id_tensor.name if nc.partition_id_tensor else None
    )
    in_names, out_names, out_avals, zero_outs = [], [], [], []
    for alloc in nc.m.functions[0].allocations:
        if not isinstance(alloc, mb.MemoryLocationSet):
            continue
        name = alloc.memorylocations[0].name
        if alloc.kind == "ExternalInput":
            if name != partition_name:
                in_names.append(name)
        elif alloc.kind == "ExternalOutput":
            shape = tuple(alloc.tensor_shape)
            npdt = np.dtype(mb.dt.np(alloc.dtype))
            out_avals.append(jax.core.ShapedArray(shape, npdt))
            out_names.append(name)
            zero_outs.append(np.zeros(shape, npdt))

    n_params = len(in_names)
    n_outs = len(out_names)
    param_names = list(in_names)
    # donated zero-init output buffers + partition id are also bass inputs
    in_names.extend(out_names)
    if partition_name is not None:
        in_names.append(partition_name)
    donate = tuple(range(n_params, n_params + n_outs))

    def _body(*args):
        operands = list(args)
        if partition_name is not None:
            operands.append(bass2jax.partition_id_tensor())
        outs = bass2jax._bass_exec_p.bind(
            *operands,
            out_avals=tuple(out_avals),
            in_names=tuple(in_names),
            out_names=tuple(out_names),
            lowering_input_output_aliases=(),
            sim_require_finite=True,
            sim_require_nnan=True,
            nc=nc,
        )
        return tuple(outs)

    devices = jax.devices()[:NCORES]
    mesh = Mesh(np.asarray(devices), ("core",))
    in_specs = (PartitionSpec("core"),) * (n_params + n_outs)
    out_specs = (PartitionSpec("core"),) * n_outs
    fn = jax.jit(
        shard_map(
            _body, mesh=mesh, in_specs=in_specs, out_specs=out_specs,
            check_rep=False,
        ),
        donate_argnums=donate,
        keep_unused=True,
    )

    def run(in_maps):
        concat_in = [
            np.concatenate([in_maps[c][n] for c in range(NCORES)], axis=0)
            for n in param_names
        ]
        concat_zeros = [
            np.zeros((NCORES * z.shape[0], *z.shape[1:]), z.dtype)
            for z in zero_outs
        ]
        out_arrs = fn(*concat_in, *concat_zeros)
        return [
            {
                n: np.asarray(out_arrs[i]).reshape(
                    NCORES, *out_avals[i].shape
                )[c]
                for i, n in enumerate(out_names)
            }
            for c in range(NCORES)
        ]

    _RUNNER = run
    return _RUNNER


def _in_maps(x, y):
    idf = np.eye(128, dtype=np.float32)
    idh = np.eye(128, dtype=np.float16)
    maps = []
    for k in range(NCORES):
        b, h = divmod(k, 2)
        xs = x[b, h * HALF:(h + 1) * HALF]    # [4096, 3]
        ys = y[b]                              # [8192, 3]
        maps.append({
            "xT": np.ascontiguousarray(xs.T),
            "yT": np.ascontiguousarray(ys.T),
            "xq": np.ascontiguousarray(
                xs.reshape(NIT, 128, 3).transpose(1, 0, 2).reshape(128, NIT * 3)
            ),
            "yq": np.ascontiguousarray(
                ys.reshape(NBLK, 128, 3).transpose(1, 0, 2).reshape(128, NBLK * 3)
            ),
            "idf": idf,
            "idh": idh,
        })
    return maps


def _postprocess(results):
    rm_total = 0.0
    cham_y_total = 0.0
    vecs = []
    for k in range(NCORES):
        rm_total += float(results[k]["rm_out"].astype(np.float64).sum())
        vecs.append(results[k]["cm_out"].T.reshape(M))  # vec[j], j = blk*128+p
    for b in range(B):
        m = np.minimum(vecs[2 * b], vecs[2 * b + 1])
        cham_y_total += float(m.astype(np.float64).sum())
    out = rm_total / (B * N) + cham_y_total / (B * M)
    return np.float32(out)


def kernel(x, y):
    x = np.asarray(x, dtype=np.float32)
    y = np.asarray(y, dtype=np.float32)
    run = _get_runner()
    return _postprocess(run(_in_maps(x, y)))


# revision 8
# speedup vs baseline: 30.9531x; 14.6207x over previous
"""Chamfer distance kernel for Trainium2 (8 NeuronCores, Bass/Tile).

Problem: x [4, 8192, 3], y [4, 8192, 3] f32.
  d[b,i,j] = ||x[b,i] - y[b,j]||^2
  out = mean_b mean_i min_j d  +  mean_b mean_j min_i d   (scalar f32)

Sharding: core k handles batch b = k//2, half h = k%2 of x's N dimension.
Each core computes d for its [4096 x-rows] x [all 8192 y] block via an
augmented K=5 matmul on the TensorEngine:
    lhsT rows = [x^2, 1, x0, x1, x2],  rhs rows = [1, y^2, -2*y0, -2*y1, -2*y2]
so PSUM holds exact f32 distances. ScalarE extracts PSUM->SBUF with an
f32->fp16 cast; VectorE keeps a running row-min (free-axis fold) and a
running col-min tile [128, 8192]. Col-min partition reduction happens on
device via PE transposes + DVE reduce. The host only combines tiny per-core
outputs ([128,1] row-min sums and [128,64] col-min blocks).
"""

import numpy as np
from contextlib import ExitStack

import concourse.bass as bass
import concourse.bacc as bacc
import concourse.tile as tile
from concourse import mybir
from concourse.bass_utils import run_bass_kernel_spmd

B, N, M, D = 4, 8192, 8192, 3
NCORES = 8
HALF = N // 2            # x rows per core
NIT = HALF // 128        # 32 i-tiles
STW = 2048               # supertile width (4 PSUM banks)
NST = M // STW           # 4 supertiles per i-tile
NBLK = M // 128          # 64 col-min blocks
F32 = mybir.dt.float32
F16 = mybir.dt.float16
AX = mybir.AxisListType.X
MIN = mybir.AluOpType.min
ADD = mybir.AluOpType.add


def _build():
    nc = bacc.Bacc("TRN2", target_bir_lowering=False, num_devices=NCORES)
    xT = nc.declare_dram_parameter("xT", [3, HALF], F32, isOutput=False)
    yT = nc.declare_dram_parameter("yT", [3, M], F32, isOutput=False)
    xq = nc.declare_dram_parameter("xq", [128, NIT * 3], F32, isOutput=False)
    yq = nc.declare_dram_parameter("yq", [128, NBLK * 3], F32, isOutput=False)
    idf = nc.declare_dram_parameter("idf", [128, 128], F32, isOutput=False)
    idh = nc.declare_dram_parameter("idh", [128, 128], F16, isOutput=False)
    rm_out = nc.declare_dram_parameter("rm_out", [128, 1], F32, isOutput=True)
    cm_out = nc.declare_dram_parameter("cm_out", [128, NBLK], F32, isOutput=True)

    with ExitStack() as ctx:
        tc = ctx.enter_context(tile.TileContext(nc))
        persist = ctx.enter_context(tc.tile_pool(name="persist", bufs=1))
        PHI = persist.tile([5, HALF], F32)
        PSI = persist.tile([5, M], F32)
        cm = persist.tile([128, M], F16)
        rm_cols = persist.tile([128, NIT], F32)
        rm_sums = persist.tile([128, 1], F32)
        cmb = persist.tile([128, NBLK], F32)
        identf = persist.tile([128, 128], F32)
        identh = persist.tile([128, 128], F16)

        # Row layout (all compute-written rows start at partition 0):
        #   PHI rows: 0-2 = x,    3 = ones, 4 = x^2
        #   PSI rows: 0-2 = -2*y, 3 = y^2,  4 = ones
        nc.sync.dma_start(out=identf, in_=idf[:, :])
        nc.sync.dma_start(out=identh, in_=idh[:, :])
        nc.sync.dma_start(out=PHI[0:3, :], in_=xT[:, :])
        nc.vector.memset(cm, 60000.0)

        # ---- prep: squared norms into PHI[4] (x^2) and PSI[3] (y^2) ----
        with tc.tile_pool(name="prep", bufs=1) as prep, \
             tc.tile_pool(name="prep_ps", bufs=1, space="PSUM") as prep_ps:
            ones_stage = prep.tile([1, M], F32)
            nc.vector.memset(ones_stage, 1.0)
            nc.sync.dma_start(out=PHI[3:4, :], in_=ones_stage[0:1, 0:HALF])
            nc.sync.dma_start(out=PSI[4:5, :], in_=ones_stage)
            yst = prep.tile([3, M], F32)
            nc.sync.dma_start(out=yst, in_=yT[:, :])
            nc.vector.tensor_scalar_mul(PSI[0:3, :], yst, -2.0)
            xq_t = prep.tile([128, NIT * 3], F32)
            yq_t = prep.tile([128, NBLK * 3], F32)
            nc.sync.dma_start(out=xq_t, in_=xq[:, :])
            nc.sync.dma_start(out=yq_t, in_=yq[:, :])
            sqx = prep.tile([128, NIT * 3], F32)
            sqy = prep.tile([128, NBLK * 3], F32)
            nc.scalar.activation(sqx, xq_t, mybir.ActivationFunctionType.Square)
            nc.scalar.activation(sqy, yq_t, mybir.ActivationFunctionType.Square)
            x2q = prep.tile([128, NIT], F32)
            y2q = prep.tile([128, NBLK], F32)
            nc.vector.tensor_reduce(
                out=x2q, in_=sqx.rearrange("p (t d) -> p t d", d=3), axis=AX, op=ADD
            )
            nc.vector.tensor_reduce(
                out=y2q, in_=sqy.rearrange("p (t d) -> p t d", d=3), axis=AX, op=ADD
            )
            # transpose [128, T] -> [T, 128] so free dim becomes the point idx
            x2ps = prep_ps.tile([NIT, 128], F32)
            y2ps = prep_ps.tile([NBLK, 128], F32)
            nc.tensor.transpose(x2ps, x2q, identf)
            nc.tensor.transpose(y2ps, y2q, identf)
            x2t = prep.tile([NIT, 128], F32)
            y2t = prep.tile([NBLK, 128], F32)
            nc.scalar.copy(x2t, x2ps)
            nc.scalar.copy(y2t, y2ps)
            # relayout [T, 128] -> single row [1, T*128] (j = t*128 + p)
            nc.sync.dma_start(
                out=PHI[4:5, :].rearrange("a (t p) -> a t p", p=128), in_=x2t
            )
            nc.sync.dma_start(
                out=PSI[3:4, :].rearrange("a (t p) -> a t p", p=128), in_=y2t
            )

        # ---- main loop ----
        with tc.tile_pool(name="mm_ps", bufs=2, space="PSUM") as mm_ps, \
             tc.tile_pool(name="ext", bufs=6) as ext:
            for it in range(NIT):
                phi_it = PHI[:, it * 128:(it + 1) * 128]
                rm_run = None
                for st in range(NST):
                    ps = mm_ps.tile([128, STW], F32, tag="ps", name=f"ps_{it}_{st}")
                    for c in range(STW // 512):
                        j0 = st * STW + c * 512
                        nc.tensor.matmul(
                            ps[:, c * 512:(c + 1) * 512],
                            phi_it,
                            PSI[:, j0:j0 + 512],
                            start=True,
                            stop=True,
                        )
                    e = ext.tile([128, STW], F16, tag="e", name=f"e_{it}_{st}")
                    nc.scalar.copy(e, ps)
                    # running col-min (elementwise, fp16 2x mode)
                    cs = cm[:, st * STW:(st + 1) * STW]
                    nc.vector.tensor_tensor(out=cs, in0=cs, in1=e, op=MIN)
                    # running row-min across supertiles
                    if st == 0:
                        rm_run = e
                    else:
                        nc.vector.tensor_tensor(out=rm_run, in0=rm_run, in1=e, op=MIN)
                # fold row-min 2048 -> 256, then reduce
                nc.vector.tensor_tensor(
                    out=rm_run[:, 0:1024], in0=rm_run[:, 0:1024],
                    in1=rm_run[:, 1024:2048], op=MIN,
                )
                nc.vector.tensor_tensor(
                    out=rm_run[:, 0:512], in0=rm_run[:, 0:512],
                    in1=rm_run[:, 512:1024], op=MIN,
                )
                nc.vector.tensor_tensor(
                    out=rm_run[:, 0:256], in0=rm_run[:, 0:256],
                    in1=rm_run[:, 256:512], op=MIN,
                )
                nc.vector.tensor_reduce(
                    out=rm_cols[:, it:it + 1], in_=rm_run[:, 0:256], axis=AX, op=MIN
                )

        # ---- tails ----
        nc.vector.tensor_reduce(out=rm_sums, in_=rm_cols, axis=AX, op=ADD)
        nc.sync.dma_start(out=rm_out[:, :], in_=rm_sums)

        with tc.tile_pool(name="tail_ps", bufs=2, space="PSUM") as tail_ps:
            for bg in range(NBLK // 4):
                pt = tail_ps.tile([128, 512], F16, tag="pt", name=f"pt_{bg}")
                for q in range(4):
                    blk = bg * 4 + q
                    nc.tensor.transpose(
                        pt[:, q * 128:(q + 1) * 128],
                        cm[:, blk * 128:(blk + 1) * 128],
                        identh,
                    )
                nc.vector.tensor_reduce(
                    out=cmb[:, bg * 4:(bg + 1) * 4],
                    in_=pt.rearrange("p (q f) -> p q f", f=128),
                    axis=AX,
                    op=MIN,
                )
        nc.sync.dma_start(out=cm_out[:, :], in_=cmb)

    nc.compile()
    return nc


_NC = None


def _get_nc():
    global _NC
    if _NC is None:
        _NC = _build()
    return _NC


_RUNNER = None


def _get_runner():
    """Build the Bass program once and return a cached jitted 8-core runner.

    Mirrors bass2jax.run_bass_via_pjrt's multi-core path, but keeps the jitted
    shard_map callable alive so repeated kernel() calls skip XLA re-tracing.
    """
    global _RUNNER
    if _RUNNER is not None:
        return _RUNNER
    import jax
    from jax.sharding import Mesh, PartitionSpec
    from jax.experimental.shard_map import shard_map
    from concourse import mybir as mb
    from concourse import bass2jax

    nc = _get_nc()
    bass2jax.install_neuronx_cc_hook()
    partition_name = (
        nc.partition_id_tensor.name if nc.partition_id_tensor else None
    )
    in_names, out_names, out_avals, zero_outs = [], [], [], []
    for alloc in nc.m.functions[0].allocations:
        if not isinstance(alloc, mb.MemoryLocationSet):
            continue
        name = alloc.memorylocations[0].name
        if alloc.kind == "ExternalInput":
            if name != partition_name:
                in_names.append(name)
        elif alloc.kind == "ExternalOutput":
            shape = tuple(alloc.tensor_shape)
            npdt = np.dtype(mb.dt.np(alloc.dtype))
            out_avals.append(jax.core.ShapedArray(shape, npdt))
            out_names.append(name)
            zero_outs.append(np.zeros(shape, npdt))

    n_params = len(in_names)
    n_outs = len(out_names)
    param_names = list(in_names)
    # donated zero-init output buffers + partition id are also bass inputs
    in_names.extend(out_names)
    if partition_name is not None:
        in_names.append(partition_name)
    donate = tuple(range(n_params, n_params + n_outs))

    def _body(*args):
        operands = list(args)
        if partition_name is not None:
            operands.append(bass2jax.partition_id_tensor())
        outs = bass2jax._bass_exec_p.bind(
            *operands,
            out_avals=tuple(out_avals),
            in_names=tuple(in_names),
            out_names=tuple(out_names),
            lowering_input_output_aliases=(),
            sim_require_finite=True,
            sim_require_nnan=True,
            nc=nc,
        )
        return tuple(outs)

    devices = jax.devices()[:NCORES]
    mesh = Mesh(np.asarray(devices), ("core",))
    in_specs = (PartitionSpec("core"),) * (n_params + n_outs)
    out_specs = (PartitionSpec("core"),) * n_outs
    fn = jax.jit(
        shard_map(
            _body, mesh=mesh, in_specs=in_specs, out_specs=out_specs,
            check_rep=False,
        ),
        donate_argnums=donate,
        keep_unused=True,
    )

    def make_zeros():
        return [
            np.zeros((NCORES * z.shape[0], *z.shape[1:]), z.dtype)
            for z in zero_outs
        ]

    def run(in_maps):
        concat_in = [
            np.concatenate([in_maps[c][n] for c in range(NCORES)], axis=0)
            for n in param_names
        ]
        out_arrs = fn(*concat_in, *make_zeros())
        return [
            {
                n: np.asarray(out_arrs[i]).reshape(
                    NCORES, *out_avals[i].shape
                )[c]
                for i, n in enumerate(out_names)
            }
            for c in range(NCORES)
        ]

    run.fn = fn
    run.mesh = mesh
    run.param_names = param_names
    run.make_zeros = make_zeros
    _RUNNER = run
    return _RUNNER


def _in_maps(x, y):
    idf = np.eye(128, dtype=np.float32)
    idh = np.eye(128, dtype=np.float16)
    maps = []
    for k in range(NCORES):
        b, h = divmod(k, 2)
        xs = x[b, h * HALF:(h + 1) * HALF]    # [4096, 3]
        ys = y[b]                              # [8192, 3]
        maps.append({
            "xT": np.ascontiguousarray(xs.T),
            "yT": np.ascontiguousarray(ys.T),
            "xq": np.ascontiguousarray(
                xs.reshape(NIT, 128, 3).transpose(1, 0, 2).reshape(128, NIT * 3)
            ),
            "yq": np.ascontiguousarray(
                ys.reshape(NBLK, 128, 3).transpose(1, 0, 2).reshape(128, NBLK * 3)
            ),
            "idf": idf,
            "idh": idh,
        })
    return maps


def _postprocess(results):
    rm_total = 0.0
    cham_y_total = 0.0
    vecs = []
    for k in range(NCORES):
        rm_total += float(results[k]["rm_out"].astype(np.float64).sum())
        vecs.append(results[k]["cm_out"].T.reshape(M))  # vec[j], j = blk*128+p
    for b in range(B):
        m = np.minimum(vecs[2 * b], vecs[2 * b + 1])
        cham_y_total += float(m.astype(np.float64).sum())
    out = rm_total / (B * N) + cham_y_total / (B * M)
    return np.float32(out)


def kernel(x, y):
    x = np.asarray(x, dtype=np.float32)
    y = np.asarray(y, dtype=np.float32)
    run = _get_runner()
    return _postprocess(run(_in_maps(x, y)))


# revision 30
# speedup vs baseline: 1065.5495x; 34.4247x over previous
"""Chamfer distance kernel for Trainium2 (8 NeuronCores, Bass/Tile).

Problem: x [4, 8192, 3], y [4, 8192, 3] f32.
  d[b,i,j] = ||x[b,i] - y[b,j]||^2
  out = mean_b mean_i min_j d  +  mean_b mean_j min_i d   (scalar f32)

Sharding: core k handles batch b = k//2, half h = k%2 of x's N dimension.
Each core computes d for its [4096 x-rows] x [all 8192 y] block via an
augmented K=5 matmul: phi = [x, 1, x^2] rows vs psi = [-2y, y^2, 1] rows, so
PSUM accumulates exact distances.

PE strategy: fp32 moving operands stream at 1/4 rate and crash with
tile_position on this runtime, so inputs are split hi/lo into fp16 pairs
(x = xh + xl; the `ones` rows split exactly, so only coord*coord products
drop the ~2^-22 lo*lo term) and each product takes 3 accumulating fp16
matmuls. Four i-tiles run concurrently in disjoint 32-row strips of the PE
array (tile_position row packing), each writing its own 512-col PSUM quarter
of a [128, 2048] supertile.

ScalarE extracts PSUM->SBUF with f32->fp16 cast. VectorE: col-min via a
min-tree into a running [128, 8192] fp16 tile; row-min via tensor_scalar
(single-src 4x mode) with fused min-accum chained through scalar1. Col-min
partition reduction happens on device via PE transposes + DVE reduce. The
host only combines tiny per-core outputs.
"""

import numpy as np
from contextlib import ExitStack

import concourse.bacc as bacc
import concourse.tile as tile
from concourse import mybir

B, N, M, D = 4, 8192, 8192, 3
NCORES = 8
HALF = N // 2            # x rows per core
NIT = HALF // 128        # 32 i-tiles
NG = NIT // 4            # 8 groups of 4 strip-packed i-tiles
JW = 512                 # j columns per strip per supertile
NST = M // JW            # 16 supertiles per group
NBLK = M // 128          # 64 col-min blocks
F32 = mybir.dt.float32
F16 = mybir.dt.float16
AX = mybir.AxisListType.X
MIN = mybir.AluOpType.min
ADD = mybir.AluOpType.add
SUB = mybir.AluOpType.subtract


def _build(repeat=1, loop_n=None, nstrip=2):
    nc = bacc.Bacc("TRN2", target_bir_lowering=False, num_devices=NCORES)
    xT = nc.declare_dram_parameter("xT", [3, HALF], F32, isOutput=False)
    yT = nc.declare_dram_parameter("yT", [3, M], F32, isOutput=False)
    xq = nc.declare_dram_parameter("xq", [128, NIT * 3], F32, isOutput=False)
    yq = nc.declare_dram_parameter("yq", [128, NBLK * 3], F32, isOutput=False)
    idf = nc.declare_dram_parameter("idf", [128, 128], F32, isOutput=False)
    idh = nc.declare_dram_parameter("idh", [128, 128], F16, isOutput=False)
    rm_out = nc.declare_dram_parameter("rm_out", [128, 1], F32, isOutput=True)
    cm_out = nc.declare_dram_parameter("cm_out", [128, NBLK], F32, isOutput=True)

    with ExitStack() as ctx:
        tc = ctx.enter_context(tile.TileContext(nc))
        persist = ctx.enter_context(tc.tile_pool(name="persist", bufs=1))
        # hi/lo fp16 operand tiles, replicated in rows 32s..32s+4 per strip s
        PHIh = persist.tile([128, HALF], F16)
        PHIl = persist.tile([128, HALF], F16)
        PSIh = persist.tile([128, M], F16)
        PSIl = persist.tile([128, M], F16)
        cm = persist.tile([128, M], F16)
        rm_cols = persist.tile([128, NIT], F32)
        rm_sums = persist.tile([128, 1], F32)
        cmb = persist.tile([128, NBLK], F32)
        identf = persist.tile([128, 128], F32)
        identh = persist.tile([128, 128], F16)

        nc.sync.dma_start(out=identf, in_=idf[:, :])
        nc.sync.dma_start(out=identh, in_=idh[:, :])

        # ---- prep: build f32 phi/psi, split hi/lo, replicate to strips ----
        # Row layout: PHI rows 0-2 = x, 3 = ones, 4 = x^2
        #             PSI rows 0-2 = -2*y, 3 = y^2, 4 = ones
        with tc.tile_pool(name="prep", bufs=1) as prep, \
             tc.tile_pool(name="prep_ps", bufs=1, space="PSUM") as prep_ps:
            PHIs = prep.tile([5, HALF], F32)
            PSIs = prep.tile([5, M], F32)
            ones_stage = prep.tile([1, M], F32)
            nc.vector.memset(ones_stage, 1.0)
            nc.sync.dma_start(out=PHIs[0:3, :], in_=xT[:, :])
            nc.sync.dma_start(out=PHIs[3:4, :], in_=ones_stage[0:1, 0:HALF])
            nc.sync.dma_start(out=PSIs[4:5, :], in_=ones_stage)
            yst = prep.tile([3, M], F32)
            nc.sync.dma_start(out=yst, in_=yT[:, :])
            nc.vector.tensor_scalar_mul(PSIs[0:3, :], yst, -2.0)
            xq_t = prep.tile([128, NIT * 3], F32)
            yq_t = prep.tile([128, NBLK * 3], F32)
            nc.sync.dma_start(out=xq_t, in_=xq[:, :])
            nc.sync.dma_start(out=yq_t, in_=yq[:, :])
            sqx = prep.tile([128, NIT * 3], F32)
            sqy = prep.tile([128, NBLK * 3], F32)
            nc.scalar.activation(sqx, xq_t, mybir.ActivationFunctionType.Square)
            nc.scalar.activation(sqy, yq_t, mybir.ActivationFunctionType.Square)
            x2q = prep.tile([128, NIT], F32)
            y2q = prep.tile([128, NBLK], F32)
            nc.vector.tensor_reduce(
                out=x2q, in_=sqx.rearrange("p (t d) -> p t d", d=3), axis=AX, op=ADD
            )
            nc.vector.tensor_reduce(
                out=y2q, in_=sqy.rearrange("p (t d) -> p t d", d=3), axis=AX, op=ADD
            )
            x2ps = prep_ps.tile([NIT, 128], F32)
            y2ps = prep_ps.tile([NBLK, 128], F32)
            nc.tensor.transpose(x2ps, x2q, identf)
            nc.tensor.transpose(y2ps, y2q, identf)
            x2t = prep.tile([NIT, 128], F32)
            y2t = prep.tile([NBLK, 128], F32)
            nc.scalar.copy(x2t, x2ps)
            nc.scalar.copy(y2t, y2ps)
            nc.sync.dma_start(
                out=PHIs[4:5, :].rearrange("a (t p) -> a t p", p=128), in_=x2t
            )
            nc.sync.dma_start(
                out=PSIs[3:4, :].rearrange("a (t p) -> a t p", p=128), in_=y2t
            )
            # hi/lo split at strip 0 (h = fp16(v); l = fp16(v - h)),
            # chunked so downstream matmuls can start on early chunks
            nc.vector.tensor_copy(PHIh[0:5, :], PHIs)
            nc.vector.tensor_tensor(
                out=PHIl[0:5, :], in0=PHIs, in1=PHIh[0:5, :], op=SUB
            )
            for h in range(4):
                c = slice(h * (M // 4), (h + 1) * (M // 4))
                nc.vector.tensor_copy(PSIh[0:5, c], PSIs[:, c])
                nc.vector.tensor_tensor(
                    out=PSIl[0:5, c], in0=PSIs[:, c], in1=PSIh[0:5, c], op=SUB
                )
            # replicate strip 0 -> strips 1..3
            for s in range(1, 4):
                for t in (PHIh, PHIl, PSIh, PSIl):
                    nc.sync.dma_start(out=t[32 * s:32 * s + 5, :], in_=t[0:5, :])

        # ---- main loop ----
        mm_ps = ctx.enter_context(tc.tile_pool(name="mm_ps", bufs=2, space="PSUM"))
        ext = ctx.enter_context(tc.tile_pool(name="ext", bufs=6))
        cmt_pool = ctx.enter_context(tc.tile_pool(name="cmt", bufs=4))
        rmv_pool = ctx.enter_context(tc.tile_pool(name="rmv", bufs=16))
        if loop_n is not None:
            ctx.enter_context(
                tc.For_i(
                    0, loop_n, 1,
                    hint_engines=(
                        mybir.EngineType.DVE,
                        mybir.EngineType.Activation,
                        mybir.EngineType.PE,
                    ),
                )
            )
        PASSES = ((PHIh, PSIh), (PHIh, PSIl), (PHIl, PSIh))
        jw = 2048 // nstrip      # j columns per strip per supertile
        nst = M // jw            # supertiles per group
        ng = NIT // nstrip       # i-tile groups
        for rep in range(repeat):
            nc.gpsimd.memset(cm, 60000.0)
            for g in range(ng):
                rmv_prev = [None] * nstrip
                for st in range(nst):
                    jsl = slice(st * jw, (st + 1) * jw)
                    ps = mm_ps.tile(
                        [128, 2048], F32, tag="ps", name=f"ps_{rep}_{g}_{st}"
                    )
                    for s in range(nstrip):
                        it = g * nstrip + s
                        isl = slice(it * 128, (it + 1) * 128)
                        for p, (L, R) in enumerate(PASSES):
                            for c in range(jw // 512):
                                nc.tensor.matmul(
                                    ps[:, s * jw + c * 512:s * jw + (c + 1) * 512],
                                    L[32 * s:32 * s + 5, isl],
                                    R[32 * s:32 * s + 5,
                                      st * jw + c * 512:st * jw + (c + 1) * 512],
                                    start=(p == 0),
                                    stop=(p == 2),
                                    tile_position=(32 * s, 0),
                                )
                    e = ext.tile(
                        [128, 2048], F16, tag="e", name=f"e_{rep}_{g}_{st}"
                    )
                    dve_extract = (st == 3)
                    if dve_extract:
                        # DVE extracts this supertile (fused with row-min),
                        # freeing ACT; ~1/8 of tiles balances the engines.
                        for s in range(nstrip):
                            hsl = slice(s * jw, (s + 1) * jw)
                            tmp = rmv_pool.tile(
                                [128, 1], F32, tag="rmv",
                                name=f"rmvx_{rep}_{g}_{st}_{s}",
                            )
                            nc.vector.tensor_scalar(
                                out=e[:, hsl], in0=ps[:, hsl], scalar1=1e30,
                                scalar2=None, op0=MIN, op1=MIN, accum_out=tmp,
                            )
                            if rmv_prev[s] is None:
                                rmv_prev[s] = tmp
                            else:
                                nxt = rmv_pool.tile(
                                    [128, 1], F32, tag="rmv",
                                    name=f"rmvm_{rep}_{g}_{st}_{s}",
                                )
                                nc.vector.tensor_tensor(
                                    out=nxt, in0=rmv_prev[s], in1=tmp, op=MIN
                                )
                                rmv_prev[s] = nxt
                    else:
                        nc.scalar.copy(e, ps)
                    # col-min: fold strips (same j range) by halves, then cm
                    w = 2048
                    src_t = e
                    while w > jw:
                        half = w // 2
                        if src_t is e:
                            m1 = cmt_pool.tile(
                                [128, half], F16, tag="m1",
                                name=f"m1_{rep}_{g}_{st}",
                            )
                            nc.vector.tensor_tensor(
                                out=m1[:, 0:half], in0=e[:, 0:half],
                                in1=e[:, half:w], op=MIN,
                            )
                            src_t = m1
                        else:
                            nc.vector.tensor_tensor(
                                out=src_t[:, 0:half], in0=src_t[:, 0:half],
                                in1=src_t[:, half:w], op=MIN,
                            )
                        w = half
                    nc.vector.tensor_tensor(
                        out=cm[:, jsl], in0=cm[:, jsl], in1=src_t[:, 0:jw], op=MIN
                    )
                    # row-min per strip: fused min-accum tensor_scalar chains
                    for s in range(nstrip if not dve_extract else 0):
                        it = g * nstrip + s
                        esl = e[:, s * jw:(s + 1) * jw]
                        if st == nst - 1:
                            accum = rm_cols[:, it:it + 1]
                        else:
                            accum = rmv_pool.tile(
                                [128, 1], F32, tag="rmv",
                                name=f"rmv_{rep}_{g}_{st}_{s}",
                            )
                        nc.vector.tensor_scalar(
                            out=esl, in0=esl,
                            scalar1=1e30 if rmv_prev[s] is None else rmv_prev[s],
                            scalar2=None, op0=MIN, op1=MIN, accum_out=accum,
                        )
                        rmv_prev[s] = accum

            # ---- tails ----
            nc.vector.tensor_reduce(out=rm_sums, in_=rm_cols, axis=AX, op=ADD)
            nc.sync.dma_start(out=rm_out[:, :], in_=rm_sums)
            for bg in range(NBLK // 4):
                pt = mm_ps.tile(
                    [128, 512], F16, tag="ps", name=f"pt_{rep}_{bg}"
                )
                for q in range(4):
                    blk = bg * 4 + q
                    nc.tensor.transpose(
                        pt[:, q * 128:(q + 1) * 128],
                        cm[:, blk * 128:(blk + 1) * 128],
                        identh,
                    )
                nc.vector.tensor_reduce(
                    out=cmb[:, bg * 4:(bg + 1) * 4],
                    in_=pt.rearrange("p (q f) -> p q f", f=128),
                    axis=AX,
                    op=MIN,
                )
            nc.sync.dma_start(out=cm_out[:, :], in_=cmb)

    nc.compile()
    return nc


_NC = None


def _get_nc():
    global _NC
    if _NC is None:
        _NC = _build()
    return _NC


_RUNNER = None


def _get_runner():
    """Build the Bass program once and return a cached jitted 8-core runner.

    Mirrors bass2jax.run_bass_via_pjrt's multi-core path, but keeps the jitted
    shard_map callable alive so repeated kernel() calls skip XLA re-tracing.
    """
    global _RUNNER
    if _RUNNER is not None:
        return _RUNNER
    import jax
    from jax.sharding import Mesh, PartitionSpec
    from jax.experimental.shard_map import shard_map
    from concourse import mybir as mb
    from concourse import bass2jax

    nc = _get_nc()
    bass2jax.install_neuronx_cc_hook()
    partition_name = (
        nc.partition_id_tensor.name if nc.partition_id_tensor else None
    )
    in_names, out_names, out_avals, zero_outs = [], [], [], []
    for alloc in nc.m.functions[0].allocations:
        if not isinstance(alloc, mb.MemoryLocationSet):
            continue
        name = alloc.memorylocations[0].name
        if alloc.kind == "ExternalInput":
            if name != partition_name:
                in_names.append(name)
        elif alloc.kind == "ExternalOutput":
            shape = tuple(alloc.tensor_shape)
            npdt = np.dtype(mb.dt.np(alloc.dtype))
            out_avals.append(jax.core.ShapedArray(shape, npdt))
            out_names.append(name)
            zero_outs.append(np.zeros(shape, npdt))

    n_params = len(in_names)
    n_outs = len(out_names)
    param_names = list(in_names)
    # donated zero-init output buffers + partition id are also bass inputs
    in_names.extend(out_names)
    if partition_name is not None:
        in_names.append(partition_name)
    donate = tuple(range(n_params, n_params + n_outs))

    def _body(*args):
        operands = list(args)
        if partition_name is not None:
            operands.append(bass2jax.partition_id_tensor())
        outs = bass2jax._bass_exec_p.bind(
            *operands,
            out_avals=tuple(out_avals),
            in_names=tuple(in_names),
            out_names=tuple(out_names),
            lowering_input_output_aliases=(),
            sim_require_finite=True,
            sim_require_nnan=True,
            nc=nc,
        )
        return tuple(outs)

    devices = jax.devices()[:NCORES]
    mesh = Mesh(np.asarray(devices), ("core",))
    in_specs = (PartitionSpec("core"),) * (n_params + n_outs)
    out_specs = (PartitionSpec("core"),) * n_outs
    fn = jax.jit(
        shard_map(
            _body, mesh=mesh, in_specs=in_specs, out_specs=out_specs,
            check_rep=False,
        ),
        donate_argnums=donate,
        keep_unused=True,
    )

    def make_zeros():
        return [
            np.zeros((NCORES * z.shape[0], *z.shape[1:]), z.dtype)
            for z in zero_outs
        ]

    def run(in_maps):
        concat_in = [
            np.concatenate([in_maps[c][n] for c in range(NCORES)], axis=0)
            for n in param_names
        ]
        out_arrs = fn(*concat_in, *make_zeros())
        return [
            {
                n: np.asarray(out_arrs[i]).reshape(
                    NCORES, *out_avals[i].shape
                )[c]
                for i, n in enumerate(out_names)
            }
            for c in range(NCORES)
        ]

    run.fn = fn
    run.mesh = mesh
    run.param_names = param_names
    run.make_zeros = make_zeros
    _RUNNER = run
    return _RUNNER


def _in_maps(x, y):
    idf = np.eye(128, dtype=np.float32)
    idh = np.eye(128, dtype=np.float16)
    maps = []
    for k in range(NCORES):
        b, h = divmod(k, 2)
        xs = x[b, h * HALF:(h + 1) * HALF]    # [4096, 3]
        ys = y[b]                              # [8192, 3]
        maps.append({
            "xT": np.ascontiguousarray(xs.T),
            "yT": np.ascontiguousarray(ys.T),
            "xq": np.ascontiguousarray(
                xs.reshape(NIT, 128, 3).transpose(1, 0, 2).reshape(128, NIT * 3)
            ),
            "yq": np.ascontiguousarray(
                ys.reshape(NBLK, 128, 3).transpose(1, 0, 2).reshape(128, NBLK * 3)
            ),
            "idf": idf,
            "idh": idh,
        })
    return maps


def _postprocess(results):
    rm_total = 0.0
    cham_y_total = 0.0
    vecs = []
    for k in range(NCORES):
        rm_total += float(results[k]["rm_out"].astype(np.float64).sum())
        vecs.append(results[k]["cm_out"].T.reshape(M))  # vec[j], j = blk*128+p
    for b in range(B):
        m = np.minimum(vecs[2 * b], vecs[2 * b + 1])
        cham_y_total += float(m.astype(np.float64).sum())
    out = rm_total / (B * N) + cham_y_total / (B * M)
    return np.float32(out)


def kernel(x, y):
    x = np.asarray(x, dtype=np.float32)
    y = np.asarray(y, dtype=np.float32)
    run = _get_runner()
    return _postprocess(run(_in_maps(x, y)))
